# revision 3
# baseline (speedup 1.0000x reference)
"""Causal MHA (B=2, S=2048, D=1024, H=16, HD=64) on 8 NeuronCores — v1.

Core c = 4*b + g handles batch b, head group g (4 heads). Host sums the 4
partial output projections per batch and adds the bias.

vs the previous kernel:
  - all device tensors are fp16 (host converts); halves DMA traffic.
  - P@V is computed with P^T tiles as the stationary operand and V (augmented
    with a ones column) streaming 65-wide: out is ctx[q,64] plus the softmax
    denominator in column 64.
  - ctx[q, feat] is transposed back to [feat, q] per chunk with the DMA xbar
    (dma_start_transpose) for the output projection.
  - exp runs on 2-bank PSUM tiles to amortize activation overhead.
"""

import numpy as np

import concourse.mybir as mybir
from concourse import bacc
from concourse.tile import TileContext
from concourse.bass_utils import run_bass_kernel_spmd
from concourse.masks import make_upper_triangular, make_identity

F32 = mybir.dt.float32
FP16 = mybir.dt.float16
Exp = mybir.ActivationFunctionType.Exp
Alu = mybir.AluOpType

B, S, D, H, HD = 2, 2048, 1024, 16, 64
GH = 4            # heads per core
GD = GH * HD      # 256 features per core
N_CORES = 8
NQT = S // 128    # 16 q tiles
NC_ = S // 512    # 4 chunks


def _build():
    nc = bacc.Bacc("TRN2", target_bir_lowering=False, name="mha_tp_v1")
    xt_d = nc.dram_tensor("xt", [D, S], FP16, kind="ExternalInput")
    wq_d = nc.dram_tensor("wqT", [D, GD], FP16, kind="ExternalInput")
    wk_d = nc.dram_tensor("wkT", [D, GD], FP16, kind="ExternalInput")
    wv_d = nc.dram_tensor("wvT", [D, GD], FP16, kind="ExternalInput")
    wo_d = nc.dram_tensor("woT", [GD, D], FP16, kind="ExternalInput")
    out_d = nc.dram_tensor("out", [S, D], FP16, kind="ExternalOutput")

    with TileContext(nc) as tc:
        with (
            tc.tile_pool(name="per", bufs=1) as per,
            tc.tile_pool(name="pt", bufs=56) as ptp,
            # NB: pt slots sized exactly for S3-early liveness
            tc.tile_pool(name="wk1", bufs=4) as wk1,
            tc.tile_pool(name="qtp", bufs=2) as qtp,
            tc.tile_pool(name="ob", bufs=5) as obp,
            tc.tile_pool(name="ps_s", bufs=2, space="PSUM") as ps_s,
            tc.tile_pool(name="ps_c", bufs=2, space="PSUM") as ps_c,
            tc.tile_pool(name="ps_p", bufs=2, space="PSUM") as ps_p,
        ):
            xt = per.tile([128, 8, S], FP16)        # X^T, d-tile major
            wq = per.tile([128, 8, GD], FP16)
            wk = per.tile([128, 8, GD], FP16)
            wv = per.tile([128, 8, GD], FP16)
            wo = per.tile([128, 2, D], FP16)
            kt = per.tile([128, 2, S], FP16)
            vaug = per.tile([128, 16, 4 * (HD + 1)], FP16)  # V + ones col/head
            c01 = per.tile([128, NQT, 128], FP16)   # ctx [q, feat] heads 0,1
            c23 = per.tile([128, NQT, 128], FP16)
            ctxn = per.tile([128, 2, NQT, 128], FP16)  # ctx^T [feat, q]
            tri = per.tile([128, 128], FP16)        # tri[kk,c]=1 iff kk<=c
            ident = per.tile([128, 128], FP16)

            wtile = per.tile([128, 32], FP16)
            nc.gpsimd.memset(wtile[:, :], 0.5)
            # PE warm-up: dependency-free matmuls so the p-state ramp
            # completes before the first projection matmul arrives.
            wsp = ps_s.tile([128, 1024], F32, tag="sp")
            for _ in range(135):
                nc.tensor.matmul(wsp[0:32, 0:32], wtile[:, :], wtile[:, :],
                                 start=True, stop=True)
            make_upper_triangular(nc, tri[:, :], val=1.0, diag=True)
            make_identity(nc, ident[:, :])

            def copy_to(eng, out, in_):
                if hasattr(eng, "tensor_copy"):
                    eng.tensor_copy(out, in_)
                else:
                    eng.copy(out, in_)

            # ---- input DMA waves (rearranged HBM APs: few big DMAs) ----
            xt_r = xt_d.rearrange("(dt p) s -> p dt s", p=128)
            def dma_x(sc, eng):
                eng.dma_start(xt[:, :, 512 * sc:512 * sc + 512],
                              xt_r[:, :, 512 * sc:512 * sc + 512])

            def dma_w(dst, src, eng):
                eng.dma_start(dst[:, :, :], src.rearrange("(dt p) f -> p dt f", p=128))

            wq_r = wq_d.rearrange("(dt p) f -> p dt f", p=128)
            wk_r = wk_d.rearrange("(dt p) f -> p dt f", p=128)
            nc.sync.dma_start(wq[:, 0:4, :], wq_r[:, 0:4, :])
            nc.scalar.dma_start(xt[:, 0:2, 0:512], xt_r[:, 0:2, 0:512])
            nc.sync.dma_start(wq[:, 4:8, :], wq_r[:, 4:8, :])
            nc.scalar.dma_start(xt[:, 2:4, 0:512], xt_r[:, 2:4, 0:512])
            nc.sync.dma_start(xt[:, 4:6, 0:512], xt_r[:, 4:6, 0:512])
            nc.scalar.dma_start(xt[:, 6:8, 0:512], xt_r[:, 6:8, 0:512])
            nc.sync.dma_start(wk[:, 0:4, :], wk_r[:, 0:4, :])
            nc.scalar.dma_start(wk[:, 4:8, :], wk_r[:, 4:8, :])
            dma_x(1, nc.sync)
            dma_w(wv, wv_d, nc.scalar)
            dma_x(2, nc.sync)
            dma_x(3, nc.scalar)
            nc.sync.dma_start(wo[:, :, :], wo_d.rearrange("(dp p) f -> p dp f", p=128))

            # ---- projections ----
            qt_chunks = {}

            def emit_q(w_t, dst, sc, drains):
                """QK chunk sc: psum [feat128, 512] x2 dp; drain to fp16.
                dst None => allocate a per-chunk Q^T tile from qtp."""
                if dst is None:
                    dst = qtp.tile([128, 2, 512], FP16, tag="qt")
                    qt_chunks[sc] = dst
                    cols = slice(0, 512)
                else:
                    cols = slice(512 * sc, 512 * sc + 512)
                pss = [ps_p.tile([128, 512], F32, tag="pp", name=f"psq{dp}") for dp in range(2)]
                for dt in range(8):
                    for dp in range(2):
                        nc.tensor.matmul(
                            pss[dp][:, :],
                            w_t[:, dt, 128 * dp:128 * dp + 128],
                            xt[:, dt, 512 * sc:512 * sc + 512],
                            start=(dt == 0), stop=(dt == 7),
                        )
                for dp in range(2):
                    eng = drains[dp % len(drains)]
                    copy_to(eng, dst[:, dp, cols], pss[dp][:, :])

            def emit_v(sc, drains, sts=None):
                rng = range(4 * sc, 4 * sc + 4) if sts is None else \
                    [4 * sc + t for t in sts]
                for st in rng:
                    psv = ps_p.tile([128, 512], F32, tag="pp")
                    for dt in range(8):
                        nc.tensor.matmul(
                            psv[:, 0:256],
                            xt[:, dt, 128 * st:128 * st + 128],
                            wv[:, dt, :],
                            start=(dt == 0), stop=(dt == 7),
                        )
                    v_dst = vaug[:, st, :].rearrange("p (h c) -> p h c", c=HD + 1)
                    eng = drains[st % len(drains)]
                    copy_to(eng,
                        v_dst[:, :, 0:HD],
                        psv[:, 0:256].rearrange("p (h c) -> p h c", c=HD),
                    )
                    nc.vector.tensor_scalar(
                        v_dst[:, :, HD:HD + 1], psv[:, 0:4], 0.0, 1.0,
                        op0=Alu.mult, op1=Alu.add,
                    )

            # ---- attention ----
            # pt_map[(h, kt)] = (tile_ap, base_col) for the current chunk's
            # P^T tiles: column base_col + 128*j holds q-tile j's slice...
            # (for diag tiles the mapping is col = base + (q_local - 128*jmin))
            pt_map = {}

            def head_slices(h):
                i, qo = h // 2, (h % 2) * 64
                return i, qo

            def emit_scores_full(qc, h, kb):
                """Full k-tiles 2*kb, 2*kb+1 for chunk qc, head h."""
                i, qo = head_slices(h)
                sp = ps_s.tile([128, 1024], F32, tag="sp")
                qt_c = qt_chunks[qc]
                for j2 in range(2):
                    k_t = 2 * kb + j2
                    nc.tensor.matmul(
                        sp[:, 512 * j2:512 * j2 + 512],
                        kt[qo:qo + 64, i, 128 * k_t:128 * k_t + 128],
                        qt_c[qo:qo + 64, i, 0:512],
                        start=True, stop=True,
                    )
                pt = ptp.tile([128, 1024], FP16, tag="pt")
                nc.scalar.activation(pt[:, :], sp[:, :], Exp, scale=0.125)
                pt_map[(qc, h, 2 * kb)] = (pt, 0)
                pt_map[(qc, h, 2 * kb + 1)] = (pt, 512)

            def emit_scores_diag(qc, h, grp, mask_engs):
                """Diagonal k-tiles. grp 0: j=0,1 (widths 512,384);
                grp 1: j=2,3 (widths 256,128). Packed tight per tile."""
                i, qo = head_slices(h)
                js = (0, 1) if grp == 0 else (2, 3)
                offs = (0, 512) if grp == 0 else (0, 256)
                widths = (512, 384) if grp == 0 else (256, 128)
                tot = offs[1] + widths[1]
                sp = ps_s.tile([128, 1024], F32, tag="sp")
                qt_c = qt_chunks[qc]
                for j, off, w in zip(js, offs, widths):
                    k_t = 4 * qc + j
                    nc.tensor.matmul(
                        sp[:, off:off + w],
                        kt[qo:qo + 64, i, 128 * k_t:128 * k_t + 128],
                        qt_c[qo:qo + 64, i, 128 * j:512],
                        start=True, stop=True,
                    )
                pt = ptp.tile([128, 1024], FP16, tag="pt")
                nc.scalar.activation(pt[:, 0:tot], sp[:, 0:tot], Exp, scale=0.125)
                for n, (j, off, w) in enumerate(zip(js, offs, widths)):
                    # diagonal 128-block mask; cols off..off+128
                    eng = mask_engs[n % len(mask_engs)]
                    eng.tensor_mul(pt[:, off:off + 128], pt[:, off:off + 128], tri[:, :])
                    # q-tile j2 >= j reads cols off + 128*(j2-j)
                    pt_map[(qc, h, 4 * qc + j)] = (pt, off - 128 * j)

            def emit_pv(qc, jj, norm_engs=(nc.vector,)):
                """ctx for q-tile qt_g = 4*qc+jj: all 4 heads into one bank."""
                qt_g = 4 * qc + jj
                ctx = ps_c.tile([128, 512], F32, tag="ctx")
                for h in range(4):
                    n_kt = qt_g + 1
                    for k_t in range(n_kt):
                        pt, base = pt_map[(qc, h, k_t)]
                        col = base + 128 * jj if k_t < 4 * qc else base + 128 * jj
                        nc.tensor.matmul(
                            ctx[:, 65 * h:65 * h + 65],
                            pt[:, col:col + 128],
                            vaug[:, k_t, 65 * h:65 * h + 65],
                            start=(k_t == 0), stop=(k_t == n_kt - 1),
                        )
                # normalize: r = 1/l (cols 64::65), ctxn_q = ctx * r
                r4 = wk1.tile([128, 4], F32, tag="r4")
                nc.vector.reciprocal_approx_fast(
                    r4[:, :], ctx[:, 64:64 + 65 * 4:65]
                )
                for h in range(4):
                    dst = c01 if h < 2 else c23
                    eng = norm_engs[h % len(norm_engs)]
                    o = dst[:, qt_g, 64 * (h % 2):64 * (h % 2) + 64]
                    i_ = ctx[:, 65 * h:65 * h + 64]
                    if hasattr(eng, "tensor_scalar"):
                        eng.tensor_scalar(o, i_, r4[:, h:h + 1], None, op0=Alu.mult)
                    else:
                        eng.mul(o, i_, r4[:, h:h + 1])

            def emit_pe_transpose(qt_g, drains):
                for dp, src in enumerate((c01, c23)):
                    tp = ps_s.tile([128, 128], FP16, tag="sp")
                    nc.tensor.transpose(tp[:, :], src[:, qt_g, :], ident[:, :])
                    eng = drains[dp % len(drains)]
                    copy_to(eng, ctxn[:, dp, qt_g, :], tp[:, :])

            def emit_transpose(qc, eng, half=None):
                lo, n = (4 * qc, 4) if half is None else (4 * qc + 2 * half, 2)
                for dp, src in enumerate((c01, c23)):
                    eng.dma_start_transpose(
                        ctxn[:, dp, lo:lo + n, :],
                        src[:, lo:lo + n, :],
                    )

            def emit_outproj(qc, drains, split_last=False, half=None, rev=False,
                             sts=None):
                if sts is None:
                    sts = range(4 * qc, 4 * qc + 4) if half is None else \
                        range(4 * qc + 2 * half, 4 * qc + 2 * half + 2)
                sts = list(sts)[::-1] if rev else list(sts)
                for st in sts:
                    ob = obp.tile([128, 1024], FP16, tag="ob")
                    split = split_last
                    for oc in range(2):
                        pso = ps_p.tile([128, 512], F32, tag="pp")
                        for dp in range(2):
                            nc.tensor.matmul(
                                pso[:, :],
                                ctxn[:, dp, st, :],
                                wo[:, dp, 512 * oc:512 * oc + 512],
                                start=(dp == 0), stop=(dp == 1),
                            )
                        eng = drains[oc % len(drains)]
                        copy_to(eng, ob[:, 512 * oc:512 * oc + 512], pso[:, :])
                        if split:
                            nc.sync.dma_start(
                                out_d[128 * st:128 * st + 128, 512 * oc:512 * oc + 512],
                                ob[:, 512 * oc:512 * oc + 512])
                    if not split:
                        nc.sync.dma_start(out_d[128 * st:128 * st + 128, :], ob[:, :])

            V = nc.vector
            A = nc.scalar
            G = nc.gpsimd

            def emit_scores(qc, mask_engs, kb_lo=0, kb_hi=None, diag=True):
                hi = 2 * qc if kb_hi is None else kb_hi
                for kb in range(kb_lo, hi):
                    for h in range(4):
                        emit_scores_full(qc, h, kb)
                if diag:
                    for h in range(4):
                        emit_scores_diag(qc, h, 0, mask_engs)
                    for h in range(4):
                        emit_scores_diag(qc, h, 1, mask_engs)

            def emit_pv_all(qc, norm_engs=(nc.vector,)):
                for jj in range(4):
                    emit_pv(qc, jj, norm_engs)

            # ---------------- schedule ----------------
            emit_q(wq, None, 0, (V, A))
            emit_q(wk, kt, 0, (A, V))
            emit_scores(0, (V, G))
            emit_q(wq, None, 1, (V, A))
            emit_q(wk, kt, 1, (A, V))
            emit_scores(1, (G, V))
            emit_v(0, (V,))
            emit_q(wq, None, 2, (V,))
            emit_q(wk, kt, 2, (V,))
            emit_scores(2, (G, V))
            emit_v(1, (V,))
            emit_pv_all(0)
            emit_q(wq, None, 3, (V,))
            emit_q(wk, kt, 3, (V,))
            emit_scores(3, (G, V), kb_lo=0, kb_hi=4, diag=False)
            emit_transpose(0, nc.sync)
            emit_outproj(0, (V,))
            emit_v(2, (V,))
            emit_pv_all(1)
            emit_transpose(1, nc.sync)
            emit_scores(3, (G, V), kb_lo=4, kb_hi=6, diag=True)
            emit_outproj(1, (V,))
            emit_v(3, (V,))
            emit_pv_all(2)
            emit_transpose(2, nc.sync)
            emit_pv(3, 0, (V, A))
            emit_outproj(2, (V,), half=0)
            emit_pv(3, 1, (A, V))
            emit_pe_transpose(12, (V, A)); emit_pe_transpose(13, (A, V))
            emit_outproj(2, (V,), half=1)
            emit_pv(3, 2, (V, A)); emit_pv(3, 3, (A, V))
            emit_outproj(3, (A, V), half=0)
            emit_pe_transpose(14, (V, A)); emit_pe_transpose(15, (A, V))
            emit_outproj(3, (A, V), split_last=True, half=1, rev=True)
    nc.compile()
    return nc


_NC = None


def _get_nc():
    global _NC
    if _NC is None:
        _NC = _build()
    return _NC


def kernel(**inputs):
    x = np.asarray(inputs["inputs"], dtype=np.float32)
    wq = np.asarray(inputs["Wq"], dtype=np.float32)
    wk = np.asarray(inputs["Wk"], dtype=np.float32)
    wv = np.asarray(inputs["Wv"], dtype=np.float32)
    wo = np.asarray(inputs["Wo"], dtype=np.float32)
    bo = np.asarray(inputs["bo"], dtype=np.float32)

    xts = [np.ascontiguousarray(x[b].T).astype(np.float16) for b in range(B)]
    in_maps = []
    for c in range(N_CORES):
        b, g = c // 4, c % 4
        sl = slice(GD * g, GD * g + GD)
        in_maps.append({
            "xt": xts[b],
            "wqT": np.ascontiguousarray(wq[sl, :].T.astype(np.float16)),
            "wkT": np.ascontiguousarray(wk[sl, :].T.astype(np.float16)),
            "wvT": np.ascontiguousarray(wv[sl, :].T.astype(np.float16)),
            "woT": np.ascontiguousarray(wo[:, sl].T.astype(np.float16)),
        })

    nc = _get_nc()
    res = run_bass_kernel_spmd(nc, in_maps, core_ids=list(range(N_CORES)))
    out = np.empty((B, S, D), np.float32)
    for b in range(B):
        acc = res.results[4 * b + 0]["out"].astype(np.float32)
        for g in range(1, 4):
            acc = acc + res.results[4 * b + g]["out"].astype(np.float32)
        out[b] = acc + bo
    return out


# revision 5
# speedup vs baseline: 1.0077x; 1.0077x over previous
"""Causal MHA (B=2, S=2048, D=1024, H=16, HD=64) on 8 NeuronCores — v1.

Core c = 4*b + g handles batch b, head group g (4 heads). Host sums the 4
partial output projections per batch and adds the bias.

vs the previous kernel:
  - all device tensors are fp16 (host converts); halves DMA traffic.
  - P@V is computed with P^T tiles as the stationary operand and V (augmented
    with a ones column) streaming 65-wide: out is ctx[q,64] plus the softmax
    denominator in column 64.
  - ctx[q, feat] is transposed back to [feat, q] per chunk with the DMA xbar
    (dma_start_transpose) for the output projection.
  - exp runs on 2-bank PSUM tiles to amortize activation overhead.
"""

import numpy as np

import concourse.mybir as mybir
from concourse import bacc
from concourse.tile import TileContext
from concourse.bass_utils import run_bass_kernel_spmd
from concourse.masks import make_upper_triangular, make_identity

F32 = mybir.dt.float32
FP16 = mybir.dt.float16
Exp = mybir.ActivationFunctionType.Exp
Alu = mybir.AluOpType

B, S, D, H, HD = 2, 2048, 1024, 16, 64
GH = 4            # heads per core
GD = GH * HD      # 256 features per core
N_CORES = 8
NQT = S // 128    # 16 q tiles
NC_ = S // 512    # 4 chunks


def _build():
    nc = bacc.Bacc("TRN2", target_bir_lowering=False, name="mha_tp_v1")
    xt_d = nc.dram_tensor("xt", [D, S], FP16, kind="ExternalInput")
    wq_d = nc.dram_tensor("wqT", [D, GD], FP16, kind="ExternalInput")
    wk_d = nc.dram_tensor("wkT", [D, GD], FP16, kind="ExternalInput")
    wv_d = nc.dram_tensor("wvT", [D, GD], FP16, kind="ExternalInput")
    wo_d = nc.dram_tensor("woT", [GD, D], FP16, kind="ExternalInput")
    out_d = nc.dram_tensor("out", [S, D], FP16, kind="ExternalOutput")

    with TileContext(nc) as tc:
        with (
            tc.tile_pool(name="per", bufs=1) as per,
            tc.tile_pool(name="pt", bufs=56) as ptp,
            # NB: pt slots sized exactly for S3-early liveness
            tc.tile_pool(name="wk1", bufs=4) as wk1,
            tc.tile_pool(name="qtp", bufs=2) as qtp,
            tc.tile_pool(name="ob", bufs=5) as obp,
            tc.tile_pool(name="ps_s", bufs=2, space="PSUM") as ps_s,
            tc.tile_pool(name="ps_c", bufs=2, space="PSUM") as ps_c,
            tc.tile_pool(name="ps_p", bufs=2, space="PSUM") as ps_p,
        ):
            xt = per.tile([128, 8, S], FP16)        # X^T, d-tile major
            wq = per.tile([128, 8, GD], FP16)
            wk = per.tile([128, 8, GD], FP16)
            wv = per.tile([128, 8, GD], FP16)
            wo = per.tile([128, 2, D], FP16)
            kt = per.tile([128, 2, S], FP16)
            vaug = per.tile([128, 16, 4 * (HD + 1)], FP16)  # V + ones col/head
            c01 = per.tile([128, NQT, 128], FP16)   # ctx [q, feat] heads 0,1
            c23 = per.tile([128, NQT, 128], FP16)
            ctxn = per.tile([128, 2, NQT, 128], FP16)  # ctx^T [feat, q]
            tri = per.tile([128, 128], FP16)        # tri[kk,c]=1 iff kk<=c
            ident = per.tile([128, 128], FP16)

            wtile = per.tile([128, 32], FP16)
            nc.gpsimd.memset(wtile[:, :], 0.5)
            va_r = vaug.rearrange("p a (h c) -> p a h c", c=HD + 1)
            nc.gpsimd.memset(va_r[:, :, :, HD], 1.0)
            # PE warm-up: dependency-free matmuls so the p-state ramp
            # completes before the first projection matmul arrives.
            wsp = ps_s.tile([128, 1024], F32, tag="sp")
            for _ in range(145):
                nc.tensor.matmul(wsp[0:32, 0:32], wtile[:, :], wtile[:, :],
                                 start=True, stop=True)
            make_upper_triangular(nc, tri[:, :], val=1.0, diag=True)
            make_identity(nc, ident[:, :])

            def copy_to(eng, out, in_):
                if hasattr(eng, "tensor_copy"):
                    eng.tensor_copy(out, in_)
                else:
                    eng.copy(out, in_)

            # ---- input DMA waves (rearranged HBM APs: few big DMAs) ----
            xt_r = xt_d.rearrange("(dt p) s -> p dt s", p=128)
            def dma_x(sc, eng):
                eng.dma_start(xt[:, :, 512 * sc:512 * sc + 512],
                              xt_r[:, :, 512 * sc:512 * sc + 512])

            def dma_w(dst, src, eng):
                eng.dma_start(dst[:, :, :], src.rearrange("(dt p) f -> p dt f", p=128))

            wq_r = wq_d.rearrange("(dt p) f -> p dt f", p=128)
            wk_r = wk_d.rearrange("(dt p) f -> p dt f", p=128)
            nc.sync.dma_start(wq[:, 0:4, :], wq_r[:, 0:4, :])
            nc.scalar.dma_start(xt[:, 0:2, 0:512], xt_r[:, 0:2, 0:512])
            nc.sync.dma_start(wq[:, 4:8, :], wq_r[:, 4:8, :])
            nc.scalar.dma_start(xt[:, 2:4, 0:512], xt_r[:, 2:4, 0:512])
            nc.sync.dma_start(xt[:, 4:6, 0:512], xt_r[:, 4:6, 0:512])
            nc.scalar.dma_start(xt[:, 6:8, 0:512], xt_r[:, 6:8, 0:512])
            nc.sync.dma_start(wk[:, 0:4, :], wk_r[:, 0:4, :])
            nc.scalar.dma_start(wk[:, 4:8, :], wk_r[:, 4:8, :])
            dma_x(1, nc.sync)
            dma_w(wv, wv_d, nc.scalar)
            dma_x(2, nc.sync)
            dma_x(3, nc.scalar)
            nc.sync.dma_start(wo[:, :, :], wo_d.rearrange("(dp p) f -> p dp f", p=128))

            # ---- projections ----
            qt_chunks = {}

            def emit_q(w_t, dst, sc, drains):
                """QK chunk sc: psum [feat128, 512] x2 dp; drain to fp16.
                dst None => allocate a per-chunk Q^T tile from qtp."""
                if dst is None:
                    dst = qtp.tile([128, 2, 512], FP16, tag="qt")
                    qt_chunks[sc] = dst
                    cols = slice(0, 512)
                else:
                    cols = slice(512 * sc, 512 * sc + 512)
                pss = [ps_p.tile([128, 512], F32, tag="pp", name=f"psq{dp}") for dp in range(2)]
                for dt in range(8):
                    for dp in range(2):
                        nc.tensor.matmul(
                            pss[dp][:, :],
                            w_t[:, dt, 128 * dp:128 * dp + 128],
                            xt[:, dt, 512 * sc:512 * sc + 512],
                            start=(dt == 0), stop=(dt == 7),
                        )
                for dp in range(2):
                    eng = drains[dp % len(drains)]
                    copy_to(eng, dst[:, dp, cols], pss[dp][:, :])

            def emit_v(sc, drains, sts=None):
                rng = range(4 * sc, 4 * sc + 4) if sts is None else \
                    [4 * sc + t for t in sts]
                for st in rng:
                    psv = ps_p.tile([128, 512], F32, tag="pp")
                    for dt in range(8):
                        nc.tensor.matmul(
                            psv[:, 0:256],
                            xt[:, dt, 128 * st:128 * st + 128],
                            wv[:, dt, :],
                            start=(dt == 0), stop=(dt == 7),
                        )
                    v_dst = vaug[:, st, :].rearrange("p (h c) -> p h c", c=HD + 1)
                    eng = drains[st % len(drains)]
                    copy_to(eng,
                        v_dst[:, :, 0:HD],
                        psv[:, 0:256].rearrange("p (h c) -> p h c", c=HD),
                    )

            # ---- attention ----
            # pt_map[(h, kt)] = (tile_ap, base_col) for the current chunk's
            # P^T tiles: column base_col + 128*j holds q-tile j's slice...
            # (for diag tiles the mapping is col = base + (q_local - 128*jmin))
            pt_map = {}

            def head_slices(h):
                i, qo = h // 2, (h % 2) * 64
                return i, qo

            def emit_scores_full(qc, h, kb):
                """Full k-tiles 2*kb, 2*kb+1 for chunk qc, head h."""
                i, qo = head_slices(h)
                sp = ps_s.tile([128, 1024], F32, tag="sp")
                qt_c = qt_chunks[qc]
                for j2 in range(2):
                    k_t = 2 * kb + j2
                    nc.tensor.matmul(
                        sp[:, 512 * j2:512 * j2 + 512],
                        kt[qo:qo + 64, i, 128 * k_t:128 * k_t + 128],
                        qt_c[qo:qo + 64, i, 0:512],
                        start=True, stop=True,
                    )
                pt = ptp.tile([128, 1024], FP16, tag="pt")
                nc.scalar.activation(pt[:, :], sp[:, :], Exp, scale=0.125)
                pt_map[(qc, h, 2 * kb)] = (pt, 0)
                pt_map[(qc, h, 2 * kb + 1)] = (pt, 512)

            def emit_scores_diag(qc, h, grp, mask_engs):
                """Diagonal k-tiles. grp 0: j=0,1 (widths 512,384);
                grp 1: j=2,3 (widths 256,128). Packed tight per tile."""
                i, qo = head_slices(h)
                js = (0, 1) if grp == 0 else (2, 3)
                offs = (0, 512) if grp == 0 else (0, 256)
                widths = (512, 384) if grp == 0 else (256, 128)
                tot = offs[1] + widths[1]
                sp = ps_s.tile([128, 1024], F32, tag="sp")
                qt_c = qt_chunks[qc]
                for j, off, w in zip(js, offs, widths):
                    k_t = 4 * qc + j
                    nc.tensor.matmul(
                        sp[:, off:off + w],
                        kt[qo:qo + 64, i, 128 * k_t:128 * k_t + 128],
                        qt_c[qo:qo + 64, i, 128 * j:512],
                        start=True, stop=True,
                    )
                pt = ptp.tile([128, 1024], FP16, tag="pt")
                nc.scalar.activation(pt[:, 0:tot], sp[:, 0:tot], Exp, scale=0.125)
                for n, (j, off, w) in enumerate(zip(js, offs, widths)):
                    # diagonal 128-block mask; cols off..off+128
                    eng = mask_engs[n % len(mask_engs)]
                    eng.tensor_mul(pt[:, off:off + 128], pt[:, off:off + 128], tri[:, :])
                    # q-tile j2 >= j reads cols off + 128*(j2-j)
                    pt_map[(qc, h, 4 * qc + j)] = (pt, off - 128 * j)

            def emit_pv(qc, jj, norm_engs=(nc.vector,)):
                """ctx for q-tile qt_g = 4*qc+jj: all 4 heads into one bank."""
                qt_g = 4 * qc + jj
                ctx = ps_c.tile([128, 512], F32, tag="ctx")
                for h in range(4):
                    n_kt = qt_g + 1
                    for k_t in range(n_kt):
                        pt, base = pt_map[(qc, h, k_t)]
                        col = base + 128 * jj if k_t < 4 * qc else base + 128 * jj
                        nc.tensor.matmul(
                            ctx[:, 65 * h:65 * h + 65],
                            pt[:, col:col + 128],
                            vaug[:, k_t, 65 * h:65 * h + 65],
                            start=(k_t == 0), stop=(k_t == n_kt - 1),
                        )
                # normalize: r = 1/l (cols 64::65), ctxn_q = ctx * r
                r4 = wk1.tile([128, 4], F32, tag="r4")
                nc.vector.reciprocal_approx_fast(
                    r4[:, :], ctx[:, 64:64 + 65 * 4:65]
                )
                for h in range(4):
                    dst = c01 if h < 2 else c23
                    eng = norm_engs[h % len(norm_engs)]
                    o = dst[:, qt_g, 64 * (h % 2):64 * (h % 2) + 64]
                    i_ = ctx[:, 65 * h:65 * h + 64]
                    if hasattr(eng, "tensor_scalar"):
                        eng.tensor_scalar(o, i_, r4[:, h:h + 1], None, op0=Alu.mult)
                    else:
                        eng.mul(o, i_, r4[:, h:h + 1])

            def emit_pe_transpose(qt_g, drains):
                for dp, src in enumerate((c01, c23)):
                    tp = ps_s.tile([128, 128], FP16, tag="sp")
                    nc.tensor.transpose(tp[:, :], src[:, qt_g, :], ident[:, :])
                    eng = drains[dp % len(drains)]
                    copy_to(eng, ctxn[:, dp, qt_g, :], tp[:, :])

            def emit_transpose(qc, eng, half=None):
                lo, n = (4 * qc, 4) if half is None else (4 * qc + 2 * half, 2)
                for dp, src in enumerate((c01, c23)):
                    eng.dma_start_transpose(
                        ctxn[:, dp, lo:lo + n, :],
                        src[:, lo:lo + n, :],
                    )

            def emit_outproj(qc, drains, split_last=False, half=None, rev=False,
                             sts=None):
                if sts is None:
                    sts = range(4 * qc, 4 * qc + 4) if half is None else \
                        range(4 * qc + 2 * half, 4 * qc + 2 * half + 2)
                sts = list(sts)[::-1] if rev else list(sts)
                for st in sts:
                    ob = obp.tile([128, 1024], FP16, tag="ob")
                    split = split_last
                    for oc in range(2):
                        pso = ps_p.tile([128, 512], F32, tag="pp")
                        for dp in range(2):
                            nc.tensor.matmul(
                                pso[:, :],
                                ctxn[:, dp, st, :],
                                wo[:, dp, 512 * oc:512 * oc + 512],
                                start=(dp == 0), stop=(dp == 1),
                            )
                        eng = drains[oc % len(drains)]
                        copy_to(eng, ob[:, 512 * oc:512 * oc + 512], pso[:, :])
                        if split:
                            nc.sync.dma_start(
                                out_d[128 * st:128 * st + 128, 512 * oc:512 * oc + 512],
                                ob[:, 512 * oc:512 * oc + 512])
                    if not split:
                        nc.sync.dma_start(out_d[128 * st:128 * st + 128, :], ob[:, :])

            V = nc.vector
            A = nc.scalar
            G = nc.gpsimd

            def emit_scores(qc, mask_engs, kb_lo=0, kb_hi=None, diag=True):
                hi = 2 * qc if kb_hi is None else kb_hi
                for kb in range(kb_lo, hi):
                    for h in range(4):
                        emit_scores_full(qc, h, kb)
                if diag:
                    for h in range(4):
                        emit_scores_diag(qc, h, 0, mask_engs)
                    for h in range(4):
                        emit_scores_diag(qc, h, 1, mask_engs)

            def emit_pv_all(qc, norm_engs=(nc.vector,)):
                for jj in range(4):
                    emit_pv(qc, jj, norm_engs)

            # ---------------- schedule ----------------
            emit_q(wq, None, 0, (V, A))
            emit_q(wk, kt, 0, (A, V))
            emit_scores(0, (V, G))
            emit_q(wq, None, 1, (V, A))
            emit_q(wk, kt, 1, (A, V))
            emit_scores(1, (G, V))
            emit_v(0, (V,))
            emit_q(wq, None, 2, (V,))
            emit_q(wk, kt, 2, (V,))
            emit_scores(2, (G, V))
            emit_v(1, (V,))
            emit_pv_all(0)
            emit_q(wq, None, 3, (V,))
            emit_q(wk, kt, 3, (V,))
            emit_scores(3, (G, V), kb_lo=0, kb_hi=4, diag=False)
            emit_transpose(0, nc.sync)
            emit_outproj(0, (V,))
            emit_v(2, (V,))
            emit_pv_all(1)
            emit_transpose(1, nc.sync)
            emit_v(3, (V,))
            emit_scores(3, (G, V), kb_lo=4, kb_hi=6, diag=True)
            emit_outproj(1, (V,))
            emit_pv_all(2)
            emit_transpose(2, nc.sync)
            emit_pv(3, 0, (V, A))
            emit_outproj(2, (V,), half=0)
            emit_pv(3, 1, (A, V))
            emit_pe_transpose(12, (V, A)); emit_pe_transpose(13, (A, V))
            emit_outproj(2, (V,), half=1)
            emit_pv(3, 2, (V, A)); emit_pv(3, 3, (A, V))
            emit_outproj(3, (A, V), half=0)
            emit_pe_transpose(14, (V, A)); emit_pe_transpose(15, (A, V))
            emit_outproj(3, (A, V), split_last=True, half=1, rev=True)
    nc.compile()
    return nc


_NC = None


def _get_nc():
    global _NC
    if _NC is None:
        _NC = _build()
    return _NC


def kernel(**inputs):
    x = np.asarray(inputs["inputs"], dtype=np.float32)
    wq = np.asarray(inputs["Wq"], dtype=np.float32)
    wk = np.asarray(inputs["Wk"], dtype=np.float32)
    wv = np.asarray(inputs["Wv"], dtype=np.float32)
    wo = np.asarray(inputs["Wo"], dtype=np.float32)
    bo = np.asarray(inputs["bo"], dtype=np.float32)

    xts = [np.ascontiguousarray(x[b].T).astype(np.float16) for b in range(B)]
    in_maps = []
    for c in range(N_CORES):
        b, g = c // 4, c % 4
        sl = slice(GD * g, GD * g + GD)
        in_maps.append({
            "xt": xts[b],
            "wqT": np.ascontiguousarray(wq[sl, :].T.astype(np.float16)),
            "wkT": np.ascontiguousarray(wk[sl, :].T.astype(np.float16)),
            "wvT": np.ascontiguousarray(wv[sl, :].T.astype(np.float16)),
            "woT": np.ascontiguousarray(wo[:, sl].T.astype(np.float16)),
        })

    nc = _get_nc()
    res = run_bass_kernel_spmd(nc, in_maps, core_ids=list(range(N_CORES)))
    out = np.empty((B, S, D), np.float32)
    for b in range(B):
        acc = res.results[4 * b + 0]["out"].astype(np.float32)
        for g in range(1, 4):
            acc = acc + res.results[4 * b + g]["out"].astype(np.float32)
        out[b] = acc + bo
    return out


# revision 6
# speedup vs baseline: 1.0079x; 1.0001x over previous
"""Causal MHA (B=2, S=2048, D=1024, H=16, HD=64) on 8 NeuronCores — v1.

Core c = 4*b + g handles batch b, head group g (4 heads). Host sums the 4
partial output projections per batch and adds the bias.

vs the previous kernel:
  - all device tensors are fp16 (host converts); halves DMA traffic.
  - P@V is computed with P^T tiles as the stationary operand and V (augmented
    with a ones column) streaming 65-wide: out is ctx[q,64] plus the softmax
    denominator in column 64.
  - ctx[q, feat] is transposed back to [feat, q] per chunk with the DMA xbar
    (dma_start_transpose) for the output projection.
  - exp runs on 2-bank PSUM tiles to amortize activation overhead.
"""

import numpy as np

import concourse.mybir as mybir
from concourse import bacc
from concourse.tile import TileContext
from concourse.bass_utils import run_bass_kernel_spmd
from concourse.masks import make_upper_triangular, make_identity

F32 = mybir.dt.float32
FP16 = mybir.dt.float16
Exp = mybir.ActivationFunctionType.Exp
Alu = mybir.AluOpType

B, S, D, H, HD = 2, 2048, 1024, 16, 64
GH = 4            # heads per core
GD = GH * HD      # 256 features per core
N_CORES = 8
NQT = S // 128    # 16 q tiles
NC_ = S // 512    # 4 chunks


def _build():
    nc = bacc.Bacc("TRN2", target_bir_lowering=False, name="mha_tp_v1")
    xt_d = nc.dram_tensor("xt", [D, S], FP16, kind="ExternalInput")
    wq_d = nc.dram_tensor("wqT", [D, GD], FP16, kind="ExternalInput")
    wk_d = nc.dram_tensor("wkT", [D, GD], FP16, kind="ExternalInput")
    wv_d = nc.dram_tensor("wvT", [D, GD], FP16, kind="ExternalInput")
    wo_d = nc.dram_tensor("woT", [GD, D], FP16, kind="ExternalInput")
    out_d = nc.dram_tensor("out", [S, D], FP16, kind="ExternalOutput")

    with TileContext(nc) as tc:
        with (
            tc.tile_pool(name="per", bufs=1) as per,
            tc.tile_pool(name="pt", bufs=56) as ptp,
            # NB: pt slots sized exactly for S3-early liveness
            tc.tile_pool(name="wk1", bufs=4) as wk1,
            tc.tile_pool(name="qtp", bufs=2) as qtp,
            tc.tile_pool(name="ob", bufs=5) as obp,
            tc.tile_pool(name="ps_s", bufs=2, space="PSUM") as ps_s,
            tc.tile_pool(name="ps_c", bufs=2, space="PSUM") as ps_c,
            tc.tile_pool(name="ps_p", bufs=2, space="PSUM") as ps_p,
        ):
            xt = per.tile([128, 8, S], FP16)        # X^T, d-tile major
            wq = per.tile([128, 8, GD], FP16)
            wk = per.tile([128, 8, GD], FP16)
            wv = per.tile([128, 8, GD], FP16)
            wo = per.tile([128, 2, D], FP16)
            kt = per.tile([128, 2, S], FP16)
            vaug = per.tile([128, 16, 4 * (HD + 1)], FP16)  # V + ones col/head
            c01 = per.tile([128, NQT, 128], FP16)   # ctx [q, feat] heads 0,1
            c23 = per.tile([128, NQT, 128], FP16)
            ctxn = per.tile([128, 2, NQT, 128], FP16)  # ctx^T [feat, q]
            tri = per.tile([128, 128], FP16)        # tri[kk,c]=1 iff kk<=c
            ident = per.tile([128, 128], FP16)

            wtile = per.tile([128, 32], FP16)
            nc.gpsimd.memset(wtile[:, :], 0.5)
            va_r = vaug.rearrange("p a (h c) -> p a h c", c=HD + 1)
            nc.gpsimd.memset(va_r[:, :, :, HD], 1.0)
            # PE warm-up: dependency-free matmuls so the p-state ramp
            # completes before the first projection matmul arrives.
            wsp = ps_s.tile([128, 1024], F32, tag="sp")
            for _ in range(145):
                nc.tensor.matmul(wsp[0:32, 0:32], wtile[:, :], wtile[:, :],
                                 start=True, stop=True)
            make_upper_triangular(nc, tri[:, :], val=1.0, diag=True)
            make_identity(nc, ident[:, :])

            def copy_to(eng, out, in_):
                if hasattr(eng, "tensor_copy"):
                    eng.tensor_copy(out, in_)
                else:
                    eng.copy(out, in_)

            # ---- input DMA waves (rearranged HBM APs: few big DMAs) ----
            xt_r = xt_d.rearrange("(dt p) s -> p dt s", p=128)
            def dma_x(sc, eng):
                eng.dma_start(xt[:, :, 512 * sc:512 * sc + 512],
                              xt_r[:, :, 512 * sc:512 * sc + 512])

            def dma_w(dst, src, eng):
                eng.dma_start(dst[:, :, :], src.rearrange("(dt p) f -> p dt f", p=128))

            wq_r = wq_d.rearrange("(dt p) f -> p dt f", p=128)
            wk_r = wk_d.rearrange("(dt p) f -> p dt f", p=128)
            nc.sync.dma_start(wq[:, 0:4, :], wq_r[:, 0:4, :])
            nc.scalar.dma_start(xt[:, 0:2, 0:512], xt_r[:, 0:2, 0:512])
            nc.sync.dma_start(wq[:, 4:8, :], wq_r[:, 4:8, :])
            nc.scalar.dma_start(xt[:, 2:4, 0:512], xt_r[:, 2:4, 0:512])
            nc.sync.dma_start(xt[:, 4:6, 0:512], xt_r[:, 4:6, 0:512])
            nc.scalar.dma_start(xt[:, 6:8, 0:512], xt_r[:, 6:8, 0:512])
            nc.sync.dma_start(wk[:, 0:4, :], wk_r[:, 0:4, :])
            nc.scalar.dma_start(wk[:, 4:8, :], wk_r[:, 4:8, :])
            dma_x(1, nc.sync)
            dma_w(wv, wv_d, nc.scalar)
            dma_x(2, nc.sync)
            dma_x(3, nc.scalar)
            nc.sync.dma_start(wo[:, :, :], wo_d.rearrange("(dp p) f -> p dp f", p=128))

            # ---- projections ----
            qt_chunks = {}

            def emit_q(w_t, dst, sc, drains):
                """QK chunk sc: psum [feat128, 512] x2 dp; drain to fp16.
                dst None => allocate a per-chunk Q^T tile from qtp."""
                if dst is None:
                    dst = qtp.tile([128, 2, 512], FP16, tag="qt")
                    qt_chunks[sc] = dst
                    cols = slice(0, 512)
                else:
                    cols = slice(512 * sc, 512 * sc + 512)
                pss = [ps_p.tile([128, 512], F32, tag="pp", name=f"psq{dp}") for dp in range(2)]
                for dt in range(8):
                    for dp in range(2):
                        nc.tensor.matmul(
                            pss[dp][:, :],
                            w_t[:, dt, 128 * dp:128 * dp + 128],
                            xt[:, dt, 512 * sc:512 * sc + 512],
                            start=(dt == 0), stop=(dt == 7),
                        )
                for dp in range(2):
                    eng = drains[dp % len(drains)]
                    copy_to(eng, dst[:, dp, cols], pss[dp][:, :])

            def emit_v(sc, drains, sts=None):
                rng = range(4 * sc, 4 * sc + 4) if sts is None else \
                    [4 * sc + t for t in sts]
                for st in rng:
                    psv = ps_p.tile([128, 512], F32, tag="pp")
                    for dt in range(8):
                        nc.tensor.matmul(
                            psv[:, 0:256],
                            xt[:, dt, 128 * st:128 * st + 128],
                            wv[:, dt, :],
                            start=(dt == 0), stop=(dt == 7),
                        )
                    v_dst = vaug[:, st, :].rearrange("p (h c) -> p h c", c=HD + 1)
                    eng = drains[st % len(drains)]
                    copy_to(eng,
                        v_dst[:, :, 0:HD],
                        psv[:, 0:256].rearrange("p (h c) -> p h c", c=HD),
                    )

            # ---- attention ----
            # pt_map[(h, kt)] = (tile_ap, base_col) for the current chunk's
            # P^T tiles: column base_col + 128*j holds q-tile j's slice...
            # (for diag tiles the mapping is col = base + (q_local - 128*jmin))
            pt_map = {}

            def head_slices(h):
                i, qo = h // 2, (h % 2) * 64
                return i, qo

            def emit_scores_full(qc, h, kb):
                """Full k-tiles 2*kb, 2*kb+1 for chunk qc, head h."""
                i, qo = head_slices(h)
                sp = ps_s.tile([128, 1024], F32, tag="sp")
                qt_c = qt_chunks[qc]
                for j2 in range(2):
                    k_t = 2 * kb + j2
                    nc.tensor.matmul(
                        sp[:, 512 * j2:512 * j2 + 512],
                        kt[qo:qo + 64, i, 128 * k_t:128 * k_t + 128],
                        qt_c[qo:qo + 64, i, 0:512],
                        start=True, stop=True,
                    )
                pt = ptp.tile([128, 1024], FP16, tag="pt")
                nc.scalar.activation(pt[:, :], sp[:, :], Exp, scale=0.125)
                pt_map[(qc, h, 2 * kb)] = (pt, 0)
                pt_map[(qc, h, 2 * kb + 1)] = (pt, 512)

            def emit_scores_diag(qc, h, grp, mask_engs):
                """Diagonal k-tiles. grp 0: j=0,1 (widths 512,384);
                grp 1: j=2,3 (widths 256,128). Packed tight per tile."""
                i, qo = head_slices(h)
                js = (0, 1) if grp == 0 else (2, 3)
                offs = (0, 512) if grp == 0 else (0, 256)
                widths = (512, 384) if grp == 0 else (256, 128)
                tot = offs[1] + widths[1]
                sp = ps_s.tile([128, 1024], F32, tag="sp")
                qt_c = qt_chunks[qc]
                for j, off, w in zip(js, offs, widths):
                    k_t = 4 * qc + j
                    nc.tensor.matmul(
                        sp[:, off:off + w],
                        kt[qo:qo + 64, i, 128 * k_t:128 * k_t + 128],
                        qt_c[qo:qo + 64, i, 128 * j:512],
                        start=True, stop=True,
                    )
                pt = ptp.tile([128, 1024], FP16, tag="pt")
                nc.scalar.activation(pt[:, 0:tot], sp[:, 0:tot], Exp, scale=0.125)
                for n, (j, off, w) in enumerate(zip(js, offs, widths)):
                    # diagonal 128-block mask; cols off..off+128
                    eng = mask_engs[n % len(mask_engs)]
                    eng.tensor_mul(pt[:, off:off + 128], pt[:, off:off + 128], tri[:, :])
                    # q-tile j2 >= j reads cols off + 128*(j2-j)
                    pt_map[(qc, h, 4 * qc + j)] = (pt, off - 128 * j)

            def emit_pv(qc, jj, norm_engs=(nc.vector,)):
                """ctx for q-tile qt_g = 4*qc+jj: all 4 heads into one bank."""
                qt_g = 4 * qc + jj
                ctx = ps_c.tile([128, 512], F32, tag="ctx")
                for h in range(4):
                    n_kt = qt_g + 1
                    for k_t in range(n_kt):
                        pt, base = pt_map[(qc, h, k_t)]
                        col = base + 128 * jj if k_t < 4 * qc else base + 128 * jj
                        nc.tensor.matmul(
                            ctx[:, 65 * h:65 * h + 65],
                            pt[:, col:col + 128],
                            vaug[:, k_t, 65 * h:65 * h + 65],
                            start=(k_t == 0), stop=(k_t == n_kt - 1),
                        )
                # normalize: r = 1/l (cols 64::65), ctxn_q = ctx * r
                r4 = wk1.tile([128, 4], F32, tag="r4")
                nc.vector.reciprocal_approx_fast(
                    r4[:, :], ctx[:, 64:64 + 65 * 4:65]
                )
                for h in range(4):
                    dst = c01 if h < 2 else c23
                    eng = norm_engs[h % len(norm_engs)]
                    o = dst[:, qt_g, 64 * (h % 2):64 * (h % 2) + 64]
                    i_ = ctx[:, 65 * h:65 * h + 64]
                    if hasattr(eng, "tensor_scalar"):
                        eng.tensor_scalar(o, i_, r4[:, h:h + 1], None, op0=Alu.mult)
                    else:
                        eng.mul(o, i_, r4[:, h:h + 1])

            def emit_pe_transpose(qt_g, drains):
                for dp, src in enumerate((c01, c23)):
                    tp = ps_s.tile([128, 128], FP16, tag="sp")
                    nc.tensor.transpose(tp[:, :], src[:, qt_g, :], ident[:, :])
                    eng = drains[dp % len(drains)]
                    copy_to(eng, ctxn[:, dp, qt_g, :], tp[:, :])

            def emit_transpose(qc, eng, half=None):
                lo, n = (4 * qc, 4) if half is None else (4 * qc + 2 * half, 2)
                for dp, src in enumerate((c01, c23)):
                    eng.dma_start_transpose(
                        ctxn[:, dp, lo:lo + n, :],
                        src[:, lo:lo + n, :],
                    )

            def emit_outproj(qc, drains, split_last=False, half=None, rev=False,
                             sts=None):
                if sts is None:
                    sts = range(4 * qc, 4 * qc + 4) if half is None else \
                        range(4 * qc + 2 * half, 4 * qc + 2 * half + 2)
                sts = list(sts)[::-1] if rev else list(sts)
                for st in sts:
                    ob = obp.tile([128, 1024], FP16, tag="ob")
                    split = split_last
                    for oc in range(2):
                        pso = ps_p.tile([128, 512], F32, tag="pp")
                        for dp in range(2):
                            nc.tensor.matmul(
                                pso[:, :],
                                ctxn[:, dp, st, :],
                                wo[:, dp, 512 * oc:512 * oc + 512],
                                start=(dp == 0), stop=(dp == 1),
                            )
                        eng = drains[oc % len(drains)]
                        copy_to(eng, ob[:, 512 * oc:512 * oc + 512], pso[:, :])
                        if split:
                            nc.sync.dma_start(
                                out_d[128 * st:128 * st + 128, 512 * oc:512 * oc + 512],
                                ob[:, 512 * oc:512 * oc + 512])
                    if not split:
                        nc.sync.dma_start(out_d[128 * st:128 * st + 128, :], ob[:, :])

            V = nc.vector
            A = nc.scalar
            G = nc.gpsimd

            def emit_scores(qc, mask_engs, kb_lo=0, kb_hi=None, diag=True):
                hi = 2 * qc if kb_hi is None else kb_hi
                for kb in range(kb_lo, hi):
                    for h in range(4):
                        emit_scores_full(qc, h, kb)
                if diag:
                    for h in range(4):
                        emit_scores_diag(qc, h, 0, mask_engs)
                    for h in range(4):
                        emit_scores_diag(qc, h, 1, mask_engs)

            def emit_pv_all(qc, norm_engs=(nc.vector,)):
                for jj in range(4):
                    emit_pv(qc, jj, norm_engs)

            # ---------------- schedule ----------------
            emit_q(wq, None, 0, (V, A))
            emit_q(wk, kt, 0, (A, V))
            emit_scores(0, (V, G))
            emit_q(wq, None, 1, (V, A))
            emit_q(wk, kt, 1, (A, V))
            emit_scores(1, (G, V))
            emit_v(0, (V,))
            emit_q(wq, None, 2, (V,))
            emit_q(wk, kt, 2, (V,))
            emit_scores(2, (G, V))
            emit_v(1, (V,))
            emit_pv_all(0)
            emit_q(wq, None, 3, (V,))
            emit_q(wk, kt, 3, (V,))
            emit_scores(3, (G, V), kb_lo=0, kb_hi=4, diag=False)
            emit_transpose(0, nc.sync)
            emit_outproj(0, (V,))
            emit_v(2, (V,))
            emit_pv_all(1)
            emit_transpose(1, nc.sync)
            emit_v(3, (V,))
            emit_scores(3, (G, V), kb_lo=4, kb_hi=6, diag=True)
            emit_outproj(1, (V,))
            emit_pv_all(2)
            emit_transpose(2, nc.sync)
            emit_pv(3, 0, (V, A))
            emit_outproj(2, (V,), half=0)
            emit_pv(3, 1, (A, V))
            emit_pe_transpose(12, (V, A)); emit_pe_transpose(13, (A, V))
            emit_outproj(2, (V,), half=1)
            emit_pv(3, 2, (V, A)); emit_pv(3, 3, (A, V))
            emit_outproj(3, (A, V), half=0)
            emit_pe_transpose(14, (V, A)); emit_pe_transpose(15, (A, V))
            # final two s-tiles, oc-interleaved for a tighter drain/DMA tail
            obs = {st: obp.tile([128, 1024], FP16, tag="ob", name=f"obf{st}")
                   for st in (15, 14)}
            for oc in range(2):
                for st in (15, 14):
                    pso = ps_p.tile([128, 512], F32, tag="pp", name=f"pf{st}{oc}")
                    for dp in range(2):
                        nc.tensor.matmul(
                            pso[:, :],
                            ctxn[:, dp, st, :],
                            wo[:, dp, 512 * oc:512 * oc + 512],
                            start=(dp == 0), stop=(dp == 1),
                        )
                    eng = (A, V)[(oc + st) % 2]
                    copy_to(eng, obs[st][:, 512 * oc:512 * oc + 512], pso[:, :])
                    nc.sync.dma_start(
                        out_d[128 * st:128 * st + 128, 512 * oc:512 * oc + 512],
                        obs[st][:, 512 * oc:512 * oc + 512])
    nc.compile()
    return nc


_NC = None


def _get_nc():
    global _NC
    if _NC is None:
        _NC = _build()
    return _NC


def kernel(**inputs):
    x = np.asarray(inputs["inputs"], dtype=np.float32)
    wq = np.asarray(inputs["Wq"], dtype=np.float32)
    wk = np.asarray(inputs["Wk"], dtype=np.float32)
    wv = np.asarray(inputs["Wv"], dtype=np.float32)
    wo = np.asarray(inputs["Wo"], dtype=np.float32)
    bo = np.asarray(inputs["bo"], dtype=np.float32)

    xts = [np.ascontiguousarray(x[b].T).astype(np.float16) for b in range(B)]
    in_maps = []
    for c in range(N_CORES):
        b, g = c // 4, c % 4
        sl = slice(GD * g, GD * g + GD)
        in_maps.append({
            "xt": xts[b],
            "wqT": np.ascontiguousarray(wq[sl, :].T.astype(np.float16)),
            "wkT": np.ascontiguousarray(wk[sl, :].T.astype(np.float16)),
            "wvT": np.ascontiguousarray(wv[sl, :].T.astype(np.float16)),
            "woT": np.ascontiguousarray(wo[:, sl].T.astype(np.float16)),
        })

    nc = _get_nc()
    res = run_bass_kernel_spmd(nc, in_maps, core_ids=list(range(N_CORES)))
    out = np.empty((B, S, D), np.float32)
    for b in range(B):
        acc = res.results[4 * b + 0]["out"].astype(np.float32)
        for g in range(1, 4):
            acc = acc + res.results[4 * b + g]["out"].astype(np.float32)
        out[b] = acc + bo
    return out


# revision 7
# speedup vs baseline: 1.0083x; 1.0004x over previous
"""Causal MHA (B=2, S=2048, D=1024, H=16, HD=64) on 8 NeuronCores — v1.

Core c = 4*b + g handles batch b, head group g (4 heads). Host sums the 4
partial output projections per batch and adds the bias.

vs the previous kernel:
  - all device tensors are fp16 (host converts); halves DMA traffic.
  - P@V is computed with P^T tiles as the stationary operand and V (augmented
    with a ones column) streaming 65-wide: out is ctx[q,64] plus the softmax
    denominator in column 64.
  - ctx[q, feat] is transposed back to [feat, q] per chunk with the DMA xbar
    (dma_start_transpose) for the output projection.
  - exp runs on 2-bank PSUM tiles to amortize activation overhead.
"""

import numpy as np

import concourse.mybir as mybir
from concourse import bacc
from concourse.tile import TileContext
from concourse.bass_utils import run_bass_kernel_spmd
from concourse.masks import make_upper_triangular, make_identity

F32 = mybir.dt.float32
FP16 = mybir.dt.float16
Exp = mybir.ActivationFunctionType.Exp
Alu = mybir.AluOpType

B, S, D, H, HD = 2, 2048, 1024, 16, 64
GH = 4            # heads per core
GD = GH * HD      # 256 features per core
N_CORES = 8
NQT = S // 128    # 16 q tiles
NC_ = S // 512    # 4 chunks


def _build():
    nc = bacc.Bacc("TRN2", target_bir_lowering=False, name="mha_tp_v1")
    xt_d = nc.dram_tensor("xt", [D, S], FP16, kind="ExternalInput")
    wq_d = nc.dram_tensor("wqT", [D, GD], FP16, kind="ExternalInput")
    wk_d = nc.dram_tensor("wkT", [D, GD], FP16, kind="ExternalInput")
    wv_d = nc.dram_tensor("wvT", [D, GD], FP16, kind="ExternalInput")
    wo_d = nc.dram_tensor("woT", [GD, D], FP16, kind="ExternalInput")
    out_d = nc.dram_tensor("out", [S, D], FP16, kind="ExternalOutput")

    with TileContext(nc) as tc:
        with (
            tc.tile_pool(name="per", bufs=1) as per,
            tc.tile_pool(name="pt", bufs=56) as ptp,
            # NB: pt slots sized exactly for S3-early liveness
            tc.tile_pool(name="wk1", bufs=4) as wk1,
            tc.tile_pool(name="qtp", bufs=2) as qtp,
            tc.tile_pool(name="ob", bufs=5) as obp,
            tc.tile_pool(name="ps_s", bufs=2, space="PSUM") as ps_s,
            tc.tile_pool(name="ps_c", bufs=2, space="PSUM") as ps_c,
            tc.tile_pool(name="ps_p", bufs=2, space="PSUM") as ps_p,
        ):
            xt = per.tile([128, 8, S], FP16)        # X^T, d-tile major
            wq = per.tile([128, 8, GD], FP16)
            wk = per.tile([128, 8, GD], FP16)
            wv = per.tile([128, 8, GD], FP16)
            wo = per.tile([128, 2, D], FP16)
            kt = per.tile([128, 2, S], FP16)
            vaug = per.tile([128, 16, 4 * (HD + 1)], FP16)  # V + ones col/head
            c01 = per.tile([128, NQT, 128], FP16)   # ctx [q, feat] heads 0,1
            c23 = per.tile([128, NQT, 128], FP16)
            ctxn = per.tile([128, 2, NQT, 128], FP16)  # ctx^T [feat, q]
            tri = per.tile([128, 128], FP16)        # tri[kk,c]=1 iff kk<=c
            ident = per.tile([128, 128], FP16)

            wtile = per.tile([128, 32], FP16)
            nc.gpsimd.memset(wtile[:, :], 0.5)
            va_r = vaug.rearrange("p a (h c) -> p a h c", c=HD + 1)
            nc.gpsimd.memset(va_r[:, :, :, HD], 1.0)
            # PE warm-up: dependency-free matmuls so the p-state ramp
            # completes before the first projection matmul arrives.
            wsp = ps_s.tile([128, 1024], F32, tag="sp")
            for _ in range(145):
                nc.tensor.matmul(wsp[0:32, 0:32], wtile[:, :], wtile[:, :],
                                 start=True, stop=True)
            make_upper_triangular(nc, tri[:, :], val=1.0, diag=True)
            make_identity(nc, ident[:, :])

            def copy_to(eng, out, in_):
                if hasattr(eng, "tensor_copy"):
                    eng.tensor_copy(out, in_)
                else:
                    eng.copy(out, in_)

            # ---- input DMA waves (rearranged HBM APs: few big DMAs) ----
            xt_r = xt_d.rearrange("(dt p) s -> p dt s", p=128)
            def dma_x(sc, eng):
                eng.dma_start(xt[:, :, 512 * sc:512 * sc + 512],
                              xt_r[:, :, 512 * sc:512 * sc + 512])

            def dma_w(dst, src, eng):
                eng.dma_start(dst[:, :, :], src.rearrange("(dt p) f -> p dt f", p=128))

            wq_r = wq_d.rearrange("(dt p) f -> p dt f", p=128)
            wk_r = wk_d.rearrange("(dt p) f -> p dt f", p=128)
            nc.sync.dma_start(wq[:, 0:4, :], wq_r[:, 0:4, :])
            nc.scalar.dma_start(xt[:, 0:2, 0:512], xt_r[:, 0:2, 0:512])
            nc.sync.dma_start(wq[:, 4:8, :], wq_r[:, 4:8, :])
            nc.scalar.dma_start(xt[:, 2:4, 0:512], xt_r[:, 2:4, 0:512])
            nc.sync.dma_start(xt[:, 4:6, 0:512], xt_r[:, 4:6, 0:512])
            nc.scalar.dma_start(xt[:, 6:8, 0:512], xt_r[:, 6:8, 0:512])
            nc.sync.dma_start(wk[:, 0:4, :], wk_r[:, 0:4, :])
            nc.scalar.dma_start(wk[:, 4:8, :], wk_r[:, 4:8, :])
            nc.sync.dma_start(xt[:, 0:4, 512:1024], xt_r[:, 0:4, 512:1024])
            nc.sync.dma_start(xt[:, 4:8, 512:1024], xt_r[:, 4:8, 512:1024])
            dma_w(wv, wv_d, nc.scalar)
            dma_x(2, nc.sync)
            dma_x(3, nc.scalar)
            nc.sync.dma_start(wo[:, :, :], wo_d.rearrange("(dp p) f -> p dp f", p=128))

            # ---- projections ----
            qt_chunks = {}

            def emit_q(w_t, dst, sc, drains):
                """QK chunk sc: psum [feat128, 512] x2 dp; drain to fp16.
                dst None => allocate a per-chunk Q^T tile from qtp."""
                if dst is None:
                    dst = qtp.tile([128, 2, 512], FP16, tag="qt")
                    qt_chunks[sc] = dst
                    cols = slice(0, 512)
                else:
                    cols = slice(512 * sc, 512 * sc + 512)
                pss = [ps_p.tile([128, 512], F32, tag="pp", name=f"psq{dp}") for dp in range(2)]
                for dt in range(8):
                    for dp in range(2):
                        nc.tensor.matmul(
                            pss[dp][:, :],
                            w_t[:, dt, 128 * dp:128 * dp + 128],
                            xt[:, dt, 512 * sc:512 * sc + 512],
                            start=(dt == 0), stop=(dt == 7),
                        )
                for dp in range(2):
                    eng = drains[dp % len(drains)]
                    copy_to(eng, dst[:, dp, cols], pss[dp][:, :])

            def emit_v(sc, drains, sts=None):
                rng = range(4 * sc, 4 * sc + 4) if sts is None else \
                    [4 * sc + t for t in sts]
                for st in rng:
                    psv = ps_p.tile([128, 512], F32, tag="pp")
                    for dt in range(8):
                        nc.tensor.matmul(
                            psv[:, 0:256],
                            xt[:, dt, 128 * st:128 * st + 128],
                            wv[:, dt, :],
                            start=(dt == 0), stop=(dt == 7),
                        )
                    v_dst = vaug[:, st, :].rearrange("p (h c) -> p h c", c=HD + 1)
                    eng = drains[st % len(drains)]
                    copy_to(eng,
                        v_dst[:, :, 0:HD],
                        psv[:, 0:256].rearrange("p (h c) -> p h c", c=HD),
                    )

            # ---- attention ----
            # pt_map[(h, kt)] = (tile_ap, base_col) for the current chunk's
            # P^T tiles: column base_col + 128*j holds q-tile j's slice...
            # (for diag tiles the mapping is col = base + (q_local - 128*jmin))
            pt_map = {}

            def head_slices(h):
                i, qo = h // 2, (h % 2) * 64
                return i, qo

            def emit_scores_full(qc, h, kb):
                """Full k-tiles 2*kb, 2*kb+1 for chunk qc, head h."""
                i, qo = head_slices(h)
                sp = ps_s.tile([128, 1024], F32, tag="sp")
                qt_c = qt_chunks[qc]
                for j2 in range(2):
                    k_t = 2 * kb + j2
                    nc.tensor.matmul(
                        sp[:, 512 * j2:512 * j2 + 512],
                        kt[qo:qo + 64, i, 128 * k_t:128 * k_t + 128],
                        qt_c[qo:qo + 64, i, 0:512],
                        start=True, stop=True,
                    )
                pt = ptp.tile([128, 1024], FP16, tag="pt")
                nc.scalar.activation(pt[:, :], sp[:, :], Exp, scale=0.125)
                pt_map[(qc, h, 2 * kb)] = (pt, 0)
                pt_map[(qc, h, 2 * kb + 1)] = (pt, 512)

            def emit_scores_diag(qc, h, grp, mask_engs):
                """Diagonal k-tiles. grp 0: j=0,1 (widths 512,384);
                grp 1: j=2,3 (widths 256,128). Packed tight per tile."""
                i, qo = head_slices(h)
                js = (0, 1) if grp == 0 else (2, 3)
                offs = (0, 512) if grp == 0 else (0, 256)
                widths = (512, 384) if grp == 0 else (256, 128)
                tot = offs[1] + widths[1]
                sp = ps_s.tile([128, 1024], F32, tag="sp")
                qt_c = qt_chunks[qc]
                for j, off, w in zip(js, offs, widths):
                    k_t = 4 * qc + j
                    nc.tensor.matmul(
                        sp[:, off:off + w],
                        kt[qo:qo + 64, i, 128 * k_t:128 * k_t + 128],
                        qt_c[qo:qo + 64, i, 128 * j:512],
                        start=True, stop=True,
                    )
                pt = ptp.tile([128, 1024], FP16, tag="pt")
                nc.scalar.activation(pt[:, 0:tot], sp[:, 0:tot], Exp, scale=0.125)
                for n, (j, off, w) in enumerate(zip(js, offs, widths)):
                    # diagonal 128-block mask; cols off..off+128
                    eng = mask_engs[n % len(mask_engs)]
                    eng.tensor_mul(pt[:, off:off + 128], pt[:, off:off + 128], tri[:, :])
                    # q-tile j2 >= j reads cols off + 128*(j2-j)
                    pt_map[(qc, h, 4 * qc + j)] = (pt, off - 128 * j)

            def emit_pv(qc, jj, norm_engs=(nc.vector,)):
                """ctx for q-tile qt_g = 4*qc+jj: all 4 heads into one bank."""
                qt_g = 4 * qc + jj
                ctx = ps_c.tile([128, 512], F32, tag="ctx")
                for h in range(4):
                    n_kt = qt_g + 1
                    for k_t in range(n_kt):
                        pt, base = pt_map[(qc, h, k_t)]
                        col = base + 128 * jj if k_t < 4 * qc else base + 128 * jj
                        nc.tensor.matmul(
                            ctx[:, 65 * h:65 * h + 65],
                            pt[:, col:col + 128],
                            vaug[:, k_t, 65 * h:65 * h + 65],
                            start=(k_t == 0), stop=(k_t == n_kt - 1),
                        )
                # normalize: r = 1/l (cols 64::65), ctxn_q = ctx * r
                r4 = wk1.tile([128, 4], F32, tag="r4")
                nc.vector.reciprocal_approx_fast(
                    r4[:, :], ctx[:, 64:64 + 65 * 4:65]
                )
                for h in range(4):
                    dst = c01 if h < 2 else c23
                    eng = norm_engs[h % len(norm_engs)]
                    o = dst[:, qt_g, 64 * (h % 2):64 * (h % 2) + 64]
                    i_ = ctx[:, 65 * h:65 * h + 64]
                    if hasattr(eng, "tensor_scalar"):
                        eng.tensor_scalar(o, i_, r4[:, h:h + 1], None, op0=Alu.mult)
                    else:
                        eng.mul(o, i_, r4[:, h:h + 1])

            def emit_pe_transpose(qt_g, drains):
                for dp, src in enumerate((c01, c23)):
                    tp = ps_s.tile([128, 128], FP16, tag="sp")
                    nc.tensor.transpose(tp[:, :], src[:, qt_g, :], ident[:, :])
                    eng = drains[dp % len(drains)]
                    copy_to(eng, ctxn[:, dp, qt_g, :], tp[:, :])

            def emit_transpose(qc, eng, half=None):
                lo, n = (4 * qc, 4) if half is None else (4 * qc + 2 * half, 2)
                for dp, src in enumerate((c01, c23)):
                    eng.dma_start_transpose(
                        ctxn[:, dp, lo:lo + n, :],
                        src[:, lo:lo + n, :],
                    )

            def emit_outproj(qc, drains, split_last=False, half=None, rev=False,
                             sts=None):
                if sts is None:
                    sts = range(4 * qc, 4 * qc + 4) if half is None else \
                        range(4 * qc + 2 * half, 4 * qc + 2 * half + 2)
                sts = list(sts)[::-1] if rev else list(sts)
                for st in sts:
                    ob = obp.tile([128, 1024], FP16, tag="ob")
                    split = split_last
                    for oc in range(2):
                        pso = ps_p.tile([128, 512], F32, tag="pp")
                        for dp in range(2):
                            nc.tensor.matmul(
                                pso[:, :],
                                ctxn[:, dp, st, :],
                                wo[:, dp, 512 * oc:512 * oc + 512],
                                start=(dp == 0), stop=(dp == 1),
                            )
                        eng = drains[oc % len(drains)]
                        copy_to(eng, ob[:, 512 * oc:512 * oc + 512], pso[:, :])
                        if split:
                            nc.sync.dma_start(
                                out_d[128 * st:128 * st + 128, 512 * oc:512 * oc + 512],
                                ob[:, 512 * oc:512 * oc + 512])
                    if not split:
                        nc.sync.dma_start(out_d[128 * st:128 * st + 128, :], ob[:, :])

            V = nc.vector
            A = nc.scalar
            G = nc.gpsimd

            def emit_scores(qc, mask_engs, kb_lo=0, kb_hi=None, diag=True):
                hi = 2 * qc if kb_hi is None else kb_hi
                for kb in range(kb_lo, hi):
                    for h in range(4):
                        emit_scores_full(qc, h, kb)
                if diag:
                    for h in range(4):
                        emit_scores_diag(qc, h, 0, mask_engs)
                    for h in range(4):
                        emit_scores_diag(qc, h, 1, mask_engs)

            def emit_pv_all(qc, norm_engs=(nc.vector,)):
                for jj in range(4):
                    emit_pv(qc, jj, norm_engs)

            # ---------------- schedule ----------------
            emit_q(wq, None, 0, (V, A))
            emit_q(wk, kt, 0, (A, V))
            emit_scores(0, (V, G))
            emit_q(wq, None, 1, (V, A))
            emit_q(wk, kt, 1, (A, V))
            emit_scores(1, (G, V))
            emit_v(0, (V,))
            emit_q(wq, None, 2, (V,))
            emit_q(wk, kt, 2, (V,))
            emit_scores(2, (G, V))
            emit_v(1, (V,))
            emit_pv_all(0)
            emit_q(wq, None, 3, (V,))
            emit_q(wk, kt, 3, (V,))
            emit_scores(3, (G, V), kb_lo=0, kb_hi=4, diag=False)
            emit_transpose(0, nc.sync)
            emit_outproj(0, (V,))
            emit_v(2, (V,))
            emit_pv_all(1)
            emit_transpose(1, nc.sync)
            emit_v(3, (V,))
            emit_scores(3, (G, V), kb_lo=4, kb_hi=6, diag=True)
            emit_outproj(1, (V,))
            emit_pv_all(2)
            emit_transpose(2, nc.sync)
            emit_pv(3, 0, (V, A))
            emit_outproj(2, (V,), half=0)
            emit_pv(3, 1, (A, V))
            emit_pe_transpose(12, (V, A)); emit_pe_transpose(13, (A, V))
            emit_outproj(2, (V,), half=1)
            emit_pv(3, 2, (V, A)); emit_pv(3, 3, (A, V))
            emit_outproj(3, (A, V), half=0)
            emit_pe_transpose(14, (V, A)); emit_pe_transpose(15, (A, V))
            # final two s-tiles, oc-interleaved for a tighter drain/DMA tail
            obs = {st: obp.tile([128, 1024], FP16, tag="ob", name=f"obf{st}")
                   for st in (15, 14)}
            for oc in range(2):
                for st in (15, 14):
                    pso = ps_p.tile([128, 512], F32, tag="pp", name=f"pf{st}{oc}")
                    for dp in range(2):
                        nc.tensor.matmul(
                            pso[:, :],
                            ctxn[:, dp, st, :],
                            wo[:, dp, 512 * oc:512 * oc + 512],
                            start=(dp == 0), stop=(dp == 1),
                        )
                    eng = (A, V)[(oc + st) % 2]
                    copy_to(eng, obs[st][:, 512 * oc:512 * oc + 512], pso[:, :])
                    nc.sync.dma_start(
                        out_d[128 * st:128 * st + 128, 512 * oc:512 * oc + 512],
                        obs[st][:, 512 * oc:512 * oc + 512])
    nc.compile()
    return nc


_NC = None


def _get_nc():
    global _NC
    if _NC is None:
        _NC = _build()
    return _NC


def kernel(**inputs):
    x = np.asarray(inputs["inputs"], dtype=np.float32)
    wq = np.asarray(inputs["Wq"], dtype=np.float32)
    wk = np.asarray(inputs["Wk"], dtype=np.float32)
    wv = np.asarray(inputs["Wv"], dtype=np.float32)
    wo = np.asarray(inputs["Wo"], dtype=np.float32)
    bo = np.asarray(inputs["bo"], dtype=np.float32)

    xts = [np.ascontiguousarray(x[b].T).astype(np.float16) for b in range(B)]
    in_maps = []
    for c in range(N_CORES):
        b, g = c // 4, c % 4
        sl = slice(GD * g, GD * g + GD)
        in_maps.append({
            "xt": xts[b],
            "wqT": np.ascontiguousarray(wq[sl, :].T.astype(np.float16)),
            "wkT": np.ascontiguousarray(wk[sl, :].T.astype(np.float16)),
            "wvT": np.ascontiguousarray(wv[sl, :].T.astype(np.float16)),
            "woT": np.ascontiguousarray(wo[:, sl].T.astype(np.float16)),
        })

    nc = _get_nc()
    res = run_bass_kernel_spmd(nc, in_maps, core_ids=list(range(N_CORES)))
    out = np.empty((B, S, D), np.float32)
    for b in range(B):
        acc = res.results[4 * b + 0]["out"].astype(np.float32)
        for g in range(1, 4):
            acc = acc + res.results[4 * b + g]["out"].astype(np.float32)
        out[b] = acc + bo
    return out


# revision 8
# speedup vs baseline: 1.0092x; 1.0009x over previous
"""Causal MHA (B=2, S=2048, D=1024, H=16, HD=64) on 8 NeuronCores — v1.

Core c = 4*b + g handles batch b, head group g (4 heads). Host sums the 4
partial output projections per batch and adds the bias.

vs the previous kernel:
  - all device tensors are fp16 (host converts); halves DMA traffic.
  - P@V is computed with P^T tiles as the stationary operand and V (augmented
    with a ones column) streaming 65-wide: out is ctx[q,64] plus the softmax
    denominator in column 64.
  - ctx[q, feat] is transposed back to [feat, q] per chunk with the DMA xbar
    (dma_start_transpose) for the output projection.
  - exp runs on 2-bank PSUM tiles to amortize activation overhead.
"""

import numpy as np

import concourse.mybir as mybir
from concourse import bacc
from concourse.tile import TileContext
from concourse.bass_utils import run_bass_kernel_spmd
from concourse.masks import make_upper_triangular, make_identity

F32 = mybir.dt.float32
FP16 = mybir.dt.float16
Exp = mybir.ActivationFunctionType.Exp
Alu = mybir.AluOpType

B, S, D, H, HD = 2, 2048, 1024, 16, 64
GH = 4            # heads per core
GD = GH * HD      # 256 features per core
N_CORES = 8
NQT = S // 128    # 16 q tiles
NC_ = S // 512    # 4 chunks


def _build():
    nc = bacc.Bacc("TRN2", target_bir_lowering=False, name="mha_tp_v1")
    xt_d = nc.dram_tensor("xt", [D, S], FP16, kind="ExternalInput")
    wq_d = nc.dram_tensor("wqT", [D, GD], FP16, kind="ExternalInput")
    wk_d = nc.dram_tensor("wkT", [D, GD], FP16, kind="ExternalInput")
    wv_d = nc.dram_tensor("wvT", [D, GD], FP16, kind="ExternalInput")
    wo_d = nc.dram_tensor("woT", [GD, D], FP16, kind="ExternalInput")
    out_d = nc.dram_tensor("out", [S, D], FP16, kind="ExternalOutput")

    with TileContext(nc) as tc:
        with (
            tc.tile_pool(name="per", bufs=1) as per,
            tc.tile_pool(name="pt", bufs=56) as ptp,
            # NB: pt slots sized exactly for S3-early liveness
            tc.tile_pool(name="wk1", bufs=4) as wk1,
            tc.tile_pool(name="qtp", bufs=2) as qtp,
            tc.tile_pool(name="ob", bufs=5) as obp,
            tc.tile_pool(name="ps_s", bufs=2, space="PSUM") as ps_s,
            tc.tile_pool(name="ps_c", bufs=2, space="PSUM") as ps_c,
            tc.tile_pool(name="ps_p", bufs=2, space="PSUM") as ps_p,
        ):
            xt = per.tile([128, 8, S], FP16)        # X^T, d-tile major
            wq = per.tile([128, 8, GD], FP16)
            wk = per.tile([128, 8, GD], FP16)
            wv = per.tile([128, 8, GD], FP16)
            wo = per.tile([128, 2, D], FP16)
            kt = per.tile([128, 2, S], FP16)
            vaug = per.tile([128, 16, 4 * (HD + 1)], FP16)  # V + ones col/head
            c01 = per.tile([128, NQT, 128], FP16)   # ctx [q, feat] heads 0,1
            c23 = per.tile([128, NQT, 128], FP16)
            ctxn = per.tile([128, 2, NQT, 128], FP16)  # ctx^T [feat, q]
            tri = per.tile([128, 128], FP16)        # tri[kk,c]=1 iff kk<=c
            ident = per.tile([128, 128], FP16)

            wtile = per.tile([128, 32], FP16)
            nc.gpsimd.memset(wtile[:, :], 0.5)
            va_r = vaug.rearrange("p a (h c) -> p a h c", c=HD + 1)
            nc.gpsimd.memset(va_r[:, :, :, HD], 1.0)
            # PE warm-up: dependency-free matmuls so the p-state ramp
            # completes before the first projection matmul arrives.
            wsp = ps_s.tile([128, 1024], F32, tag="sp")
            for _ in range(145):
                nc.tensor.matmul(wsp[0:32, 0:32], wtile[:, :], wtile[:, :],
                                 start=True, stop=True)
            make_upper_triangular(nc, tri[:, :], val=1.0, diag=True)
            make_identity(nc, ident[:, :])

            def copy_to(eng, out, in_):
                if hasattr(eng, "tensor_copy"):
                    eng.tensor_copy(out, in_)
                else:
                    eng.copy(out, in_)

            # ---- input DMA waves (rearranged HBM APs: few big DMAs) ----
            xt_r = xt_d.rearrange("(dt p) s -> p dt s", p=128)
            def dma_x(sc, eng):
                eng.dma_start(xt[:, :, 512 * sc:512 * sc + 512],
                              xt_r[:, :, 512 * sc:512 * sc + 512])

            def dma_w(dst, src, eng):
                eng.dma_start(dst[:, :, :], src.rearrange("(dt p) f -> p dt f", p=128))

            wq_r = wq_d.rearrange("(dt p) f -> p dt f", p=128)
            wk_r = wk_d.rearrange("(dt p) f -> p dt f", p=128)
            nc.sync.dma_start(wq[:, 0:4, :], wq_r[:, 0:4, :])
            nc.scalar.dma_start(xt[:, 0:2, 0:512], xt_r[:, 0:2, 0:512])
            nc.sync.dma_start(wq[:, 4:8, :], wq_r[:, 4:8, :])
            nc.scalar.dma_start(xt[:, 2:4, 0:512], xt_r[:, 2:4, 0:512])
            nc.sync.dma_start(xt[:, 4:6, 0:512], xt_r[:, 4:6, 0:512])
            nc.scalar.dma_start(xt[:, 6:8, 0:512], xt_r[:, 6:8, 0:512])
            nc.sync.dma_start(wk[:, 0:4, :], wk_r[:, 0:4, :])
            nc.scalar.dma_start(wk[:, 4:8, :], wk_r[:, 4:8, :])
            nc.sync.dma_start(xt[:, 0:4, 512:1024], xt_r[:, 0:4, 512:1024])
            nc.sync.dma_start(xt[:, 4:8, 512:1024], xt_r[:, 4:8, 512:1024])
            dma_w(wv, wv_d, nc.scalar)
            nc.sync.dma_start(xt[:, 0:4, 1024:1536], xt_r[:, 0:4, 1024:1536])
            nc.scalar.dma_start(xt[:, 0:4, 1536:2048], xt_r[:, 0:4, 1536:2048])
            nc.sync.dma_start(xt[:, 4:8, 1024:1536], xt_r[:, 4:8, 1024:1536])
            nc.scalar.dma_start(xt[:, 4:8, 1536:2048], xt_r[:, 4:8, 1536:2048])
            nc.sync.dma_start(wo[:, :, :], wo_d.rearrange("(dp p) f -> p dp f", p=128))

            # ---- projections ----
            qt_chunks = {}

            def emit_q(w_t, dst, sc, drains):
                """QK chunk sc: psum [feat128, 512] x2 dp; drain to fp16.
                dst None => allocate a per-chunk Q^T tile from qtp."""
                if dst is None:
                    dst = qtp.tile([128, 2, 512], FP16, tag="qt")
                    qt_chunks[sc] = dst
                    cols = slice(0, 512)
                else:
                    cols = slice(512 * sc, 512 * sc + 512)
                pss = [ps_p.tile([128, 512], F32, tag="pp", name=f"psq{dp}") for dp in range(2)]
                for dt in range(8):
                    for dp in range(2):
                        nc.tensor.matmul(
                            pss[dp][:, :],
                            w_t[:, dt, 128 * dp:128 * dp + 128],
                            xt[:, dt, 512 * sc:512 * sc + 512],
                            start=(dt == 0), stop=(dt == 7),
                        )
                for dp in range(2):
                    eng = drains[dp % len(drains)]
                    copy_to(eng, dst[:, dp, cols], pss[dp][:, :])

            def emit_v(sc, drains, sts=None):
                rng = range(4 * sc, 4 * sc + 4) if sts is None else \
                    [4 * sc + t for t in sts]
                for st in rng:
                    psv = ps_p.tile([128, 512], F32, tag="pp")
                    for dt in range(8):
                        nc.tensor.matmul(
                            psv[:, 0:256],
                            xt[:, dt, 128 * st:128 * st + 128],
                            wv[:, dt, :],
                            start=(dt == 0), stop=(dt == 7),
                        )
                    v_dst = vaug[:, st, :].rearrange("p (h c) -> p h c", c=HD + 1)
                    eng = drains[st % len(drains)]
                    copy_to(eng,
                        v_dst[:, :, 0:HD],
                        psv[:, 0:256].rearrange("p (h c) -> p h c", c=HD),
                    )

            # ---- attention ----
            # pt_map[(h, kt)] = (tile_ap, base_col) for the current chunk's
            # P^T tiles: column base_col + 128*j holds q-tile j's slice...
            # (for diag tiles the mapping is col = base + (q_local - 128*jmin))
            pt_map = {}

            def head_slices(h):
                i, qo = h // 2, (h % 2) * 64
                return i, qo

            def emit_scores_full(qc, h, kb):
                """Full k-tiles 2*kb, 2*kb+1 for chunk qc, head h."""
                i, qo = head_slices(h)
                sp = ps_s.tile([128, 1024], F32, tag="sp")
                qt_c = qt_chunks[qc]
                for j2 in range(2):
                    k_t = 2 * kb + j2
                    nc.tensor.matmul(
                        sp[:, 512 * j2:512 * j2 + 512],
                        kt[qo:qo + 64, i, 128 * k_t:128 * k_t + 128],
                        qt_c[qo:qo + 64, i, 0:512],
                        start=True, stop=True,
                    )
                pt = ptp.tile([128, 1024], FP16, tag="pt")
                nc.scalar.activation(pt[:, :], sp[:, :], Exp, scale=0.125)
                pt_map[(qc, h, 2 * kb)] = (pt, 0)
                pt_map[(qc, h, 2 * kb + 1)] = (pt, 512)

            def emit_scores_diag(qc, h, grp, mask_engs):
                """Diagonal k-tiles. grp 0: j=0,1 (widths 512,384);
                grp 1: j=2,3 (widths 256,128). Packed tight per tile."""
                i, qo = head_slices(h)
                js = (0, 1) if grp == 0 else (2, 3)
                offs = (0, 512) if grp == 0 else (0, 256)
                widths = (512, 384) if grp == 0 else (256, 128)
                tot = offs[1] + widths[1]
                sp = ps_s.tile([128, 1024], F32, tag="sp")
                qt_c = qt_chunks[qc]
                for j, off, w in zip(js, offs, widths):
                    k_t = 4 * qc + j
                    nc.tensor.matmul(
                        sp[:, off:off + w],
                        kt[qo:qo + 64, i, 128 * k_t:128 * k_t + 128],
                        qt_c[qo:qo + 64, i, 128 * j:512],
                        start=True, stop=True,
                    )
                pt = ptp.tile([128, 1024], FP16, tag="pt")
                nc.scalar.activation(pt[:, 0:tot], sp[:, 0:tot], Exp, scale=0.125)
                for n, (j, off, w) in enumerate(zip(js, offs, widths)):
                    # diagonal 128-block mask; cols off..off+128
                    eng = mask_engs[n % len(mask_engs)]
                    eng.tensor_mul(pt[:, off:off + 128], pt[:, off:off + 128], tri[:, :])
                    # q-tile j2 >= j reads cols off + 128*(j2-j)
                    pt_map[(qc, h, 4 * qc + j)] = (pt, off - 128 * j)

            def emit_pv(qc, jj, norm_engs=(nc.vector,)):
                """ctx for q-tile qt_g = 4*qc+jj: all 4 heads into one bank."""
                qt_g = 4 * qc + jj
                ctx = ps_c.tile([128, 512], F32, tag="ctx")
                for h in range(4):
                    n_kt = qt_g + 1
                    for k_t in range(n_kt):
                        pt, base = pt_map[(qc, h, k_t)]
                        col = base + 128 * jj if k_t < 4 * qc else base + 128 * jj
                        nc.tensor.matmul(
                            ctx[:, 65 * h:65 * h + 65],
                            pt[:, col:col + 128],
                            vaug[:, k_t, 65 * h:65 * h + 65],
                            start=(k_t == 0), stop=(k_t == n_kt - 1),
                        )
                # normalize: r = 1/l (cols 64::65), ctxn_q = ctx * r
                r4 = wk1.tile([128, 4], F32, tag="r4")
                nc.vector.reciprocal_approx_fast(
                    r4[:, :], ctx[:, 64:64 + 65 * 4:65]
                )
                for h in range(4):
                    dst = c01 if h < 2 else c23
                    eng = norm_engs[h % len(norm_engs)]
                    o = dst[:, qt_g, 64 * (h % 2):64 * (h % 2) + 64]
                    i_ = ctx[:, 65 * h:65 * h + 64]
                    if hasattr(eng, "tensor_scalar"):
                        eng.tensor_scalar(o, i_, r4[:, h:h + 1], None, op0=Alu.mult)
                    else:
                        eng.mul(o, i_, r4[:, h:h + 1])

            def emit_pe_transpose(qt_g, drains):
                for dp, src in enumerate((c01, c23)):
                    tp = ps_s.tile([128, 128], FP16, tag="sp")
                    nc.tensor.transpose(tp[:, :], src[:, qt_g, :], ident[:, :])
                    eng = drains[dp % len(drains)]
                    copy_to(eng, ctxn[:, dp, qt_g, :], tp[:, :])

            def emit_transpose(qc, eng, half=None):
                lo, n = (4 * qc, 4) if half is None else (4 * qc + 2 * half, 2)
                for dp, src in enumerate((c01, c23)):
                    eng.dma_start_transpose(
                        ctxn[:, dp, lo:lo + n, :],
                        src[:, lo:lo + n, :],
                    )

            def emit_outproj(qc, drains, split_last=False, half=None, rev=False,
                             sts=None):
                if sts is None:
                    sts = range(4 * qc, 4 * qc + 4) if half is None else \
                        range(4 * qc + 2 * half, 4 * qc + 2 * half + 2)
                sts = list(sts)[::-1] if rev else list(sts)
                for st in sts:
                    ob = obp.tile([128, 1024], FP16, tag="ob")
                    split = split_last
                    for oc in range(2):
                        pso = ps_p.tile([128, 512], F32, tag="pp")
                        for dp in range(2):
                            nc.tensor.matmul(
                                pso[:, :],
                                ctxn[:, dp, st, :],
                                wo[:, dp, 512 * oc:512 * oc + 512],
                                start=(dp == 0), stop=(dp == 1),
                            )
                        eng = drains[oc % len(drains)]
                        copy_to(eng, ob[:, 512 * oc:512 * oc + 512], pso[:, :])
                        if split:
                            nc.sync.dma_start(
                                out_d[128 * st:128 * st + 128, 512 * oc:512 * oc + 512],
                                ob[:, 512 * oc:512 * oc + 512])
                    if not split:
                        nc.sync.dma_start(out_d[128 * st:128 * st + 128, :], ob[:, :])

            V = nc.vector
            A = nc.scalar
            G = nc.gpsimd

            def emit_scores(qc, mask_engs, kb_lo=0, kb_hi=None, diag=True):
                hi = 2 * qc if kb_hi is None else kb_hi
                for kb in range(kb_lo, hi):
                    for h in range(4):
                        emit_scores_full(qc, h, kb)
                if diag:
                    for h in range(4):
                        emit_scores_diag(qc, h, 0, mask_engs)
                    for h in range(4):
                        emit_scores_diag(qc, h, 1, mask_engs)

            def emit_pv_all(qc, norm_engs=(nc.vector,)):
                for jj in range(4):
                    emit_pv(qc, jj, norm_engs)

            # ---------------- schedule ----------------
            emit_q(wq, None, 0, (V, A))
            emit_q(wk, kt, 0, (A, V))
            emit_scores(0, (V, G))
            emit_q(wq, None, 1, (V, A))
            emit_q(wk, kt, 1, (A, V))
            emit_scores(1, (G, V))
            emit_v(0, (V,))
            emit_q(wq, None, 2, (V,))
            emit_q(wk, kt, 2, (V,))
            emit_scores(2, (G, V))
            emit_v(1, (V,))
            emit_pv_all(0)
            emit_q(wq, None, 3, (V,))
            emit_q(wk, kt, 3, (V,))
            emit_scores(3, (G, V), kb_lo=0, kb_hi=4, diag=False)
            emit_transpose(0, nc.sync)
            emit_outproj(0, (V,))
            emit_v(2, (V,))
            emit_pv_all(1)
            emit_transpose(1, nc.sync)
            emit_v(3, (V,))
            emit_scores(3, (G, V), kb_lo=4, kb_hi=6, diag=True)
            emit_outproj(1, (V,))
            emit_pv_all(2)
            emit_transpose(2, nc.sync)
            emit_pv(3, 0, (V, A))
            emit_outproj(2, (V,), half=0)
            emit_pv(3, 1, (A, V))
            emit_pe_transpose(12, (V, A)); emit_pe_transpose(13, (A, V))
            emit_outproj(2, (V,), half=1)
            emit_pv(3, 2, (V, A)); emit_pv(3, 3, (A, V))
            emit_outproj(3, (A, V), half=0)
            emit_pe_transpose(14, (V, A)); emit_pe_transpose(15, (A, V))
            # final two s-tiles, oc-interleaved for a tighter drain/DMA tail
            obs = {st: obp.tile([128, 1024], FP16, tag="ob", name=f"obf{st}")
                   for st in (15, 14)}
            for oc in range(2):
                for st in (15, 14):
                    pso = ps_p.tile([128, 512], F32, tag="pp", name=f"pf{st}{oc}")
                    for dp in range(2):
                        nc.tensor.matmul(
                            pso[:, :],
                            ctxn[:, dp, st, :],
                            wo[:, dp, 512 * oc:512 * oc + 512],
                            start=(dp == 0), stop=(dp == 1),
                        )
                    eng = (A, V)[(oc + st) % 2]
                    copy_to(eng, obs[st][:, 512 * oc:512 * oc + 512], pso[:, :])
                    nc.sync.dma_start(
                        out_d[128 * st:128 * st + 128, 512 * oc:512 * oc + 512],
                        obs[st][:, 512 * oc:512 * oc + 512])
    nc.compile()
    return nc


_NC = None


def _get_nc():
    global _NC
    if _NC is None:
        _NC = _build()
    return _NC


def kernel(**inputs):
    x = np.asarray(inputs["inputs"], dtype=np.float32)
    wq = np.asarray(inputs["Wq"], dtype=np.float32)
    wk = np.asarray(inputs["Wk"], dtype=np.float32)
    wv = np.asarray(inputs["Wv"], dtype=np.float32)
    wo = np.asarray(inputs["Wo"], dtype=np.float32)
    bo = np.asarray(inputs["bo"], dtype=np.float32)

    xts = [np.ascontiguousarray(x[b].T).astype(np.float16) for b in range(B)]
    in_maps = []
    for c in range(N_CORES):
        b, g = c // 4, c % 4
        sl = slice(GD * g, GD * g + GD)
        in_maps.append({
            "xt": xts[b],
            "wqT": np.ascontiguousarray(wq[sl, :].T.astype(np.float16)),
            "wkT": np.ascontiguousarray(wk[sl, :].T.astype(np.float16)),
            "wvT": np.ascontiguousarray(wv[sl, :].T.astype(np.float16)),
            "woT": np.ascontiguousarray(wo[:, sl].T.astype(np.float16)),
        })

    nc = _get_nc()
    res = run_bass_kernel_spmd(nc, in_maps, core_ids=list(range(N_CORES)))
    out = np.empty((B, S, D), np.float32)
    for b in range(B):
        acc = res.results[4 * b + 0]["out"].astype(np.float32)
        for g in range(1, 4):
            acc = acc + res.results[4 * b + g]["out"].astype(np.float32)
        out[b] = acc + bo
    return out
